# revision 13
# baseline (speedup 1.0000x reference)
"""Trainium2 Bass kernel for triangle (AlphaFold-style) gated attention over pair rows.

Problem: B=1, N=256 rows; per row n: attention over 256 positions,
H=4 heads x CH=32, C=128 channels, additive mask bias (per row, per key),
triangle bias (per head, q, k; shared across rows), sigmoid gating,
output projection. Rows sharded across 8 NeuronCores (32 rows/core), SPMD.

v2 design (vs v1 at 195us):
  - triangle bias applied MULTIPLICATIVELY: exp(s+tri+mask) =
    exp(s+mask) * exp(tri); E = exp(tri) is row-constant, precomputed on
    host in fp16, applied by one DVE scalar_tensor_tensor (4x mode).
    Kills the per-row identity-matmul PSUM preload (2048 PE cyc/row).
  - inputs pre-cast to fp16 on host; rows loaded via DMA xbar transpose
    (dma_start_transpose DRAM->SBUF) directly into [c, tok] layout.
    Kills gpsimd SWDGE cast loads, PE transposes, and DVE psX copies.
  - scores via host-precomputed M_h = wk_h.T wq_h * scale * 256 (fp16):
    u = M.T @ xkT (k-tile chunks), sT_h = u_slice.T @ xqT (all K=128).
  - p = exp(psS/256 + mask) per kt chunk (ACT, mask is per-partition
    bias); pE = p * E in ONE fp16 4x DVE op.
  - AV + denominators as v1 (tile_position column packing, sele=2.0
    broadcast); gating fused: og = (tanh+1)*psO in one STT op.
  - fin output projection batched 2 rows per psF bank; fp16 store.
  - software pipeline: u(i+1) / main(i) / AV(i-1) / fin(i-2); PSUM
    fits exactly in 8 banks: psU 1, psB 1, psS0 2, psS1 2, psOD 1, psF 1.
"""
import numpy as np

B, N, CQ, H, CH = 1, 256, 128, 4, 32
NCORES = 8
ROWS = N // NCORES  # 32
HD = H * CH  # 128


def build_program(rows):
    import concourse.bass as bass
    import concourse.bacc as bacc
    import concourse.mybir as mybir
    from concourse import tile

    f32 = mybir.dt.float32
    fp16 = mybir.dt.float16
    AF = mybir.ActivationFunctionType
    ALU = mybir.AluOpType
    nc = bacc.Bacc("TRN2", target_bir_lowering=False, debug=False)

    qkx = nc.declare_dram_parameter("qkx", [rows, 2 * N, CQ], fp16,
                                    isOutput=False)
    maskc = nc.declare_dram_parameter("maskc", [rows, 128, 2], f32, isOutput=False)
    ecat = nc.declare_dram_parameter("ecat", [2 * H, 128, N], fp16, isOutput=False)
    mcat = nc.declare_dram_parameter("mcat", [CQ, H * CQ], fp16, isOutput=False)
    wvT = nc.declare_dram_parameter("wvT", [CQ, HD], fp16, isOutput=False)
    wgT = nc.declare_dram_parameter("wgT", [CQ, HD], fp16, isOutput=False)
    woT = nc.declare_dram_parameter("woT", [HD, CQ], fp16, isOutput=False)
    bgc = nc.declare_dram_parameter("bgc", [HD, 1], f32, isOutput=False)
    bor = nc.declare_dram_parameter("bor", [128, 2 * CQ], fp16, isOutput=False)
    sele = nc.declare_dram_parameter("sele", [128, 32], fp16, isOutput=False)
    out = nc.declare_dram_parameter("out", [rows, N, CQ], fp16, isOutput=True)

    with tile.TileContext(nc) as tc:
        with (
            nc.allow_low_precision(reason="fp16 matmul operands and "
                                   "reciprocal_approx_fast by design"),
            tc.tile_pool(name="const", bufs=1) as cp,
            tc.tile_pool(name="sb", bufs=2) as sb,
            tc.tile_pool(name="sbp", bufs=5) as sbp,
            tc.tile_pool(name="ps", bufs=1, space=bass.MemorySpace.PSUM) as ps,
        ):
            # ---- constants ----
            m_s = cp.tile([CQ, H * CQ], fp16, tag="mcat")
            wv_s = cp.tile([CQ, HD], fp16, tag="wv")
            wg_s = cp.tile([CQ, HD], fp16, tag="wg")
            wo_s = cp.tile([HD, CQ], fp16, tag="wo")
            bg_s = cp.tile([HD, 1], f32, tag="bg")
            bo_bc = cp.tile([128, 2 * CQ], fp16, tag="bo")
            sel_s = cp.tile([128, 32], fp16, tag="sele")
            e_s = cp.tile([128, 2 * H * N], fp16, tag="ecat")
            mk_all = cp.tile([128, rows, 2], f32, tag="mkall")
            for t, d in ((m_s, mcat), (wv_s, wvT), (wg_s, wgT), (wo_s, woT),
                         (bg_s, bgc), (bo_bc, bor), (sel_s, sele)):
                nc.sync.dma_start(t[:], d[:])
            for i in range(2 * H):
                nc.sync.dma_start(e_s[:, i * N:(i + 1) * N], ecat[i])
            nc.sync.dma_start(mk_all[:], maskc.rearrange("r p t -> p r t"))

            # per-row tiles are requested via helper closures so the
            # software pipeline below can reference them by row index
            xq = {}
            xk = {}
            usb = {}
            vsb = {}
            tts = {}
            pss = {}
            pes = {}
            rbs = {}
            ogs = {}
            og2s = {}

            def loads(r):
                xqk = sbp.tile([CQ, 2 * N], fp16, tag="xqk", name="xqk")
                nc.sync.dma_start_transpose(xqk[:], qkx[r])
                xq[r] = xqk[:, 0:N]
                xk[r] = xqk[:, N:2 * N]

            def u_stage01(r):
                # u_h[c', k(256)] heads 0-1 -> psU01 [128, 512]; full-width
                # moving keeps LDWEIGHTS hidden under the stream
                psU = ps.tile([128, 512], f32, tag="psU01", name="psU01")
                for h in range(2):
                    nc.tensor.matmul(psU[:, h * N:(h + 1) * N],
                                     m_s[:, h * CQ:(h + 1) * CQ],
                                     xk[r], start=True, stop=True)
                usb[r] = sb.tile([CQ, 1024], fp16, tag="usb", name="usb")
                nc.vector.tensor_copy(usb[r][:, 0:512], psU[:])

            def u_stage23(r):
                psU = ps.tile([128, 512], f32, tag="psU23", name="psU23")
                for h in range(2):
                    nc.tensor.matmul(psU[:, h * N:(h + 1) * N],
                                     m_s[:, (h + 2) * CQ:(h + 3) * CQ],
                                     xk[r], start=True, stop=True)
                nc.scalar.activation(usb[r][:, 512:1024], psU[:], AF.Copy)

            def vg_stage(r):
                psB = ps.tile([128, 512], f32, tag="psB")  # g [0:256] | v [256:512]
                nc.tensor.matmul(psB[:, 0:N], wg_s[:], xq[r],
                                 start=True, stop=True)
                for kt in range(2):
                    nc.tensor.matmul(psB[:, N + kt * 128:N + (kt + 1) * 128],
                                     xk[r][:, kt * 128:(kt + 1) * 128], wv_s[:],
                                     start=True, stop=True)
                tts[r] = sb.tile([128, N], fp16, tag="tT", name="tT")
                nc.scalar.activation(tts[r][:], psB[:, 0:N], AF.Tanh,
                                     scale=0.5, bias=bg_s[:, 0:1])
                vsb[r] = sb.tile([128, N], fp16, tag="v", name="vsb")
                nc.vector.tensor_copy(vsb[r][:], psB[:, N:2 * N])

            def scores_stage(r, kt):
                psS = ps.tile([128, H * N], f32, tag=f"psS{kt}")
                for h in range(H):
                    nc.tensor.matmul(
                        psS[:, h * N:(h + 1) * N],
                        usb[r][:, h * N + kt * 128:h * N + (kt + 1) * 128],
                        xq[r], start=True, stop=True)
                if kt == 0:
                    pss[r] = sb.tile([128, 2 * H * N], fp16, tag="ps", name="ps")
                nc.scalar.activation(pss[r][:, kt * H * N:(kt + 1) * H * N],
                                     psS[:], AF.Exp, scale=float(1.0 / 256.0),
                                     bias=mk_all[:, r, kt:kt + 1])

            def mult_stage(r, kt):
                if kt == 0:
                    pes[r] = sb.tile([128, 2 * H * N], fp16, tag="pe", name="pe")
                sl = slice(kt * H * N, (kt + 1) * H * N)
                nc.vector.tensor_mul(pes[r][:, sl], pss[r][:, sl], e_s[:, sl])

            def av_stage(r):
                psOD = ps.tile([128, 512], f32, tag="psOD")  # o [0:256]|den[256:512]
                for kt in range(2):
                    for h in range(H):
                        nc.tensor.matmul(
                            psOD[32 * h:32 * h + 32, 0:N],
                            vsb[r][:, kt * 128 + 32 * h:kt * 128 + 32 * h + 32],
                            pes[r][:, (kt * H + h) * N:(kt * H + h + 1) * N],
                            start=(kt == 0), stop=(kt == 1),
                            tile_position=(0, 32 * h), skip_group_check=True)
                for kt in range(2):
                    for h in range(H):
                        nc.tensor.matmul(
                            psOD[32 * h:32 * h + 32, N:2 * N], sel_s[:],
                            pes[r][:, (kt * H + h) * N:(kt * H + h + 1) * N],
                            start=(kt == 0), stop=(kt == 1),
                            tile_position=(0, 32 * h), skip_group_check=True)
                return psOD

            def og_a(r, psOD):
                # og = (tanh + 1) * psO  (gating, 2*sigmoid fold)
                ogs[r] = sb.tile([128, N], fp16, tag="og", name="og")
                nc.vector.scalar_tensor_tensor(
                    ogs[r][:], tts[r][:], 1.0, psOD[:, 0:N],
                    op0=ALU.add, op1=ALU.mult)

            def og_b(r, psOD):
                rbs[r] = sb.tile([128, N], f32, tag="rb", name="rb")
                nc.vector.reciprocal_approx_fast(rbs[r][:], psOD[:, N:2 * N])

            def og_c(r):
                og2s[r] = sb.tile([128, N], fp16, tag="og2", name="og2")
                nc.vector.scalar_tensor_tensor(
                    og2s[r][:], ogs[r][:], 1.0, rbs[r][:],
                    op0=ALU.mult, op1=ALU.mult)

            def fin_stage(r):
                # reuse the psB bank (g/v long since drained); bo is added by
                # the ocopy STT, so no ones/bias matmuls
                psF = ps.tile([128, 512], f32, tag="psB", name="psF")
                for qt in range(2):
                    nc.tensor.matmul(psF[:, qt * 128:(qt + 1) * 128],
                                     og2s[r][:, qt * 128:(qt + 1) * 128], wo_s[:],
                                     start=True, stop=True)
                return psF

            # ---- software pipeline ----
            loads(0)
            loads(1)
            loads(2)
            u_stage01(0)
            u_stage23(0)
            for i in range(rows + 2):
                r_load, r_next, r_cur, r_av, r_fin = i + 3, i + 1, i, i - 1, i - 2
                if r_load < rows:
                    loads(r_load)
                if r_cur < rows:
                    scores_stage(r_cur, 0)
                if r_next < rows:
                    u_stage01(r_next)   # PE u heads01 + DVE copy
                if r_cur < rows:
                    vg_stage(r_cur)     # PE vg + ACT tanh + DVE vcopy
                if 0 <= r_av < rows:
                    psOD = av_stage(r_av)   # PE AV + den (pE ready last iter)
                    og_a(r_av, psOD)        # DVE og
                if r_cur < rows:
                    mult_stage(r_cur, 0)    # DVE pE kt0 (after exp0 only)
                if r_next < rows:
                    u_stage23(r_next)   # PE u heads23 + ACT copy
                if 0 <= r_av < rows:
                    og_b(r_av, psOD)        # DVE rb
                if r_cur < rows:
                    scores_stage(r_cur, 1)
                if 0 <= r_av < rows:
                    og_c(r_av)              # DVE og2
                if r_cur < rows:
                    mult_stage(r_cur, 1)    # DVE pE kt1
                if 0 <= r_fin < rows:
                    psF = fin_stage(r_fin)
                    o_sb = sb.tile([128, N], fp16, tag="osb")
                    nc.vector.scalar_tensor_tensor(
                        o_sb[:], psF[:, 0:N], 1.0, bo_bc[:],
                        op0=ALU.mult, op1=ALU.add)
                    nc.gpsimd.dma_start(
                        out[r_fin].rearrange("(t p) c -> p t c", p=128),
                        o_sb.rearrange("p (t c) -> p t c", c=128))
                # release helper refs for long-dead rows
                for d in (xq, xk, usb, vsb, tts, pss, pes, rbs, ogs, og2s):
                    for k in [k for k in d if k < i - 3]:
                        del d[k]
    nc.compile()
    return nc


_PROG_CACHE = {}


def host_prep(q_x, kv_x, mask_bias, triangle_bias, wq, wk, wv, wg, bg, wo, bo):
    """Returns (qx fp16 [N,N,C], kvx fp16, maskc [N,128,2], shared dict)."""
    scale = np.float64(1.0 / np.float64(np.sqrt(np.float32(CH), dtype=np.float32)))
    qkx = np.concatenate(
        [np.asarray(q_x, np.float32).reshape(N, N, CQ).astype(np.float16),
         np.asarray(kv_x, np.float32).reshape(N, N, CQ).astype(np.float16)],
        axis=1)  # [N, 2N, CQ]

    wqf = np.asarray(wq, np.float64).reshape(H, CH, CQ)
    wkf = np.asarray(wk, np.float64).reshape(H, CH, CQ)
    # M_h = wk_h.T @ wq_h * scale * 256 (x256 dodges fp16 subnormals;
    # exp's scale=1/256 compensates), mcat [c, h*CQ + c']
    mcat = np.concatenate(
        [(wkf[h].T @ wqf[h] * (scale * 256.0)) for h in range(H)],
        axis=1).astype(np.float16)
    mcat = np.ascontiguousarray(mcat)
    wvT = np.ascontiguousarray(np.asarray(wv).reshape(HD, CQ).T.astype(np.float16))
    wgT = np.ascontiguousarray(np.asarray(wg).reshape(HD, CQ).T.astype(np.float16))
    woT = np.ascontiguousarray(np.asarray(wo).T.astype(np.float16))  # [e, c]
    bgc = np.ascontiguousarray(np.asarray(bg, np.float32).reshape(HD, 1) * 0.5)
    bor = np.ascontiguousarray(
        np.tile(np.asarray(bo).astype(np.float16).reshape(1, CQ), (128, 2)))
    sele = np.full((128, 32), 2.0, np.float16)
    # mask: [n, k] -> [n, k_in_tile, kt]
    m = np.asarray(mask_bias, np.float32).reshape(N, N)
    maskc = np.ascontiguousarray(m.reshape(N, 2, 128).transpose(0, 2, 1))
    # E = exp(tri): [h, q, k] -> [(kt, h), k_in_tile, q], fp16
    t = np.asarray(triangle_bias, np.float64).reshape(H, N, N)
    ecat = np.ascontiguousarray(
        np.exp(t).transpose(0, 2, 1).reshape(H, 2, 128, N).transpose(1, 0, 2, 3)
        .reshape(2 * H, 128, N).astype(np.float16))
    shared = dict(mcat=mcat, wvT=wvT, wgT=wgT, woT=woT, bgc=bgc,
                  bor=bor, sele=sele, ecat=ecat)
    return qkx, maskc, shared


def make_in_maps(q_x, kv_x, mask_bias, triangle_bias, wq, wk, wv, wg, bg, wo, bo):
    qkx, maskc, shared = host_prep(q_x, kv_x, mask_bias, triangle_bias,
                                   wq, wk, wv, wg, bg, wo, bo)
    in_maps = []
    for i in range(NCORES):
        sl = slice(i * ROWS, (i + 1) * ROWS)
        in_maps.append(dict(qkx=np.ascontiguousarray(qkx[sl]),
                            maskc=np.ascontiguousarray(maskc[sl]), **shared))
    return in_maps


def get_program():
    if ROWS not in _PROG_CACHE:
        _PROG_CACHE[ROWS] = build_program(ROWS)
    return _PROG_CACHE[ROWS]


def kernel(q_x, kv_x, mask_bias, triangle_bias, wq, wk, wv, wg, bg, wo, bo):
    from concourse.bass_utils import run_bass_kernel_spmd

    in_maps = make_in_maps(q_x, kv_x, mask_bias, triangle_bias,
                           wq, wk, wv, wg, bg, wo, bo)
    nc = get_program()
    res = run_bass_kernel_spmd(nc, in_maps, list(range(NCORES)))
    outs = [np.asarray(res.results[i]["out"]) for i in range(NCORES)]
    return np.concatenate(outs, axis=0)[None].astype(np.float32)


# revision 14
# speedup vs baseline: 1.2506x; 1.2506x over previous
"""Trainium2 Bass kernel for triangle (AlphaFold-style) gated attention over pair rows.

Problem: B=1, N=256 rows; per row n: attention over 256 positions,
H=4 heads x CH=32, C=128 channels, additive mask bias (per row, per key),
triangle bias (per head, q, k; shared across rows), sigmoid gating,
output projection. Rows sharded across 8 NeuronCores (32 rows/core), SPMD.

Design (vs v1 baseline at 195us):
  - triangle bias applied MULTIPLICATIVELY: exp(s+tri+mask) =
    exp(s+mask) * exp(tri); E = exp(tri) is row-constant, precomputed on
    host in fp16, applied by per-kt DVE tensor_mul (2x fp16 mode).
    Kills the per-row identity-matmul PSUM preload (2048 PE cyc/row).
  - inputs pre-cast to fp16 on host, qx|kvx concatenated; ONE row load
    via DMA xbar transpose (dma_start_transpose DRAM->SBUF) into
    [c, tok] layout. Kills gpsimd cast loads, PE transposes, DVE copies.
  - scores via host-precomputed M_h = wk_h.T wq_h * scale * 256 (fp16):
    u = M.T @ xkT (k-tile chunks), sT_h = u_slice.T @ xqT (all K=128).
  - p = exp(psS/256 + mask) per kt chunk (ACT, mask = per-partition
    bias); pE = p * E per-kt DVE tensor_mul so AV kt0 never waits exp1.
  - AV + denominators via tile_position column packing (4-way
    concurrent on HW), sele=2.0 broadcast; gating fused:
    og = (tanh+1)*psO in one STT op; bo folded into the output-copy STT.
  - output stores on gpsimd SWDGE queue (keeps the sync queue, which
    serially executes the 1.2us DMA transposes, from blocking loads).
  - software pipeline: loads(i+3) / u(i+1) / main(i) / AV(i-1) /
    fin(i-2, 2-row batched); PSUM = 8 banks exactly:
    psU 1, psB 1, psS0 2, psS1 2, psOD 1, psF 1.
"""
import numpy as np

B, N, CQ, H, CH = 1, 256, 128, 4, 32
NCORES = 8
ROWS = N // NCORES  # 32
HD = H * CH  # 128


def build_program(rows):
    import concourse.bass as bass
    import concourse.bacc as bacc
    import concourse.mybir as mybir
    from concourse import tile

    f32 = mybir.dt.float32
    fp16 = mybir.dt.float16
    AF = mybir.ActivationFunctionType
    ALU = mybir.AluOpType
    nc = bacc.Bacc("TRN2", target_bir_lowering=False, debug=False)

    qkx = nc.declare_dram_parameter("qkx", [rows, 2 * N, CQ], fp16,
                                    isOutput=False)
    maskc = nc.declare_dram_parameter("maskc", [rows, 128, 2], f32, isOutput=False)
    ecat = nc.declare_dram_parameter("ecat", [2 * H, 128, N], fp16, isOutput=False)
    mcat = nc.declare_dram_parameter("mcat", [CQ, H * CQ], fp16, isOutput=False)
    wvT = nc.declare_dram_parameter("wvT", [CQ, HD], fp16, isOutput=False)
    wgT = nc.declare_dram_parameter("wgT", [CQ, HD], fp16, isOutput=False)
    woT = nc.declare_dram_parameter("woT", [HD, CQ], fp16, isOutput=False)
    bgc = nc.declare_dram_parameter("bgc", [HD, 1], f32, isOutput=False)
    bor = nc.declare_dram_parameter("bor", [128, 4 * CQ], fp16, isOutput=False)
    sele = nc.declare_dram_parameter("sele", [128, 32], fp16, isOutput=False)
    out = nc.declare_dram_parameter("out", [rows, N, CQ], fp16, isOutput=True)

    with tile.TileContext(nc) as tc:
        with (
            nc.allow_low_precision(reason="fp16 matmul operands and "
                                   "reciprocal_approx_fast by design"),
            tc.tile_pool(name="const", bufs=1) as cp,
            tc.tile_pool(name="sb", bufs=2) as sb,
            tc.tile_pool(name="sbp", bufs=5) as sbp,
            tc.tile_pool(name="ps", bufs=1, space=bass.MemorySpace.PSUM) as ps,
        ):
            # ---- constants ----
            m_s = cp.tile([CQ, H * CQ], fp16, tag="mcat")
            wv_s = cp.tile([CQ, HD], fp16, tag="wv")
            wg_s = cp.tile([CQ, HD], fp16, tag="wg")
            wo_s = cp.tile([HD, CQ], fp16, tag="wo")
            bg_s = cp.tile([HD, 1], f32, tag="bg")
            bo_bc = cp.tile([128, 4 * CQ], fp16, tag="bo")
            sel_s = cp.tile([128, 32], fp16, tag="sele")
            e_s = cp.tile([128, 2 * H * N], fp16, tag="ecat")
            mk_all = cp.tile([128, rows, 2], f32, tag="mkall")
            for t, d in ((m_s, mcat), (wv_s, wvT), (wg_s, wgT), (wo_s, woT),
                         (bg_s, bgc), (bo_bc, bor), (sel_s, sele)):
                nc.sync.dma_start(t[:], d[:])
            for i in range(2 * H):
                nc.sync.dma_start(e_s[:, i * N:(i + 1) * N], ecat[i])
            nc.sync.dma_start(mk_all[:], maskc.rearrange("r p t -> p r t"))

            # per-row tiles live in dicts so the software pipeline below can
            # reference them by row index
            xq = {}
            xk = {}
            usb = {}
            vsb = {}
            tts = {}
            pss = {}
            pes = {}
            rbs = {}
            ogs = {}
            og2s = {}

            def loads(r):
                xqk = sbp.tile([CQ, 2 * N], fp16, tag="xqk", name="xqk")
                nc.sync.dma_start_transpose(xqk[:], qkx[r])
                xq[r] = xqk[:, 0:N]
                xk[r] = xqk[:, N:2 * N]

            def u_stage(r, kt):
                # u_h[c', k-tile] for 4 heads of this kt -> psU [128, 512]
                psU = ps.tile([128, 512], f32, tag="psU")
                for h in range(H):
                    nc.tensor.matmul(psU[:, h * 128:(h + 1) * 128],
                                     m_s[:, h * CQ:(h + 1) * CQ],
                                     xk[r][:, kt * 128:(kt + 1) * 128],
                                     start=True, stop=True)
                if kt == 0:
                    usb[r] = sb.tile([CQ, 1024], fp16, tag="usb", name="usb")
                    nc.vector.tensor_copy(usb[r][:, 0:512], psU[:])
                else:
                    nc.scalar.activation(usb[r][:, 512:1024], psU[:], AF.Copy)

            def vg_stage(r):
                psB = ps.tile([128, 512], f32, tag="psB")  # g [0:256] | v [256:512]
                nc.tensor.matmul(psB[:, 0:N], wg_s[:], xq[r],
                                 start=True, stop=True)
                for kt in range(2):
                    nc.tensor.matmul(psB[:, N + kt * 128:N + (kt + 1) * 128],
                                     xk[r][:, kt * 128:(kt + 1) * 128], wv_s[:],
                                     start=True, stop=True)
                tts[r] = sb.tile([128, N], fp16, tag="tT", name="tT")
                nc.scalar.activation(tts[r][:], psB[:, 0:N], AF.Tanh,
                                     scale=0.5, bias=bg_s[:, 0:1])
                vsb[r] = sb.tile([128, N], fp16, tag="v", name="vsb")
                nc.vector.tensor_copy(vsb[r][:], psB[:, N:2 * N])

            def scores_stage(r, kt):
                psS = ps.tile([128, H * N], f32, tag=f"psS{kt}")
                for h in range(H):
                    nc.tensor.matmul(
                        psS[:, h * N:(h + 1) * N],
                        usb[r][:, kt * 512 + h * 128:kt * 512 + (h + 1) * 128],
                        xq[r], start=True, stop=True)
                if kt == 0:
                    pss[r] = sb.tile([128, 2 * H * N], fp16, tag="ps", name="ps")
                nc.scalar.activation(pss[r][:, kt * H * N:(kt + 1) * H * N],
                                     psS[:], AF.Exp, scale=float(1.0 / 256.0),
                                     bias=mk_all[:, r, kt:kt + 1])

            def mult_stage(r, kt):
                if kt == 0:
                    pes[r] = sb.tile([128, 2 * H * N], fp16, tag="pe", name="pe")
                sl = slice(kt * H * N, (kt + 1) * H * N)
                nc.vector.tensor_mul(pes[r][:, sl], pss[r][:, sl], e_s[:, sl])

            def av_stage(r):
                psOD = ps.tile([128, 512], f32, tag="psOD")  # o [0:256]|den[256:512]
                for kt in range(2):
                    for h in range(H):
                        nc.tensor.matmul(
                            psOD[32 * h:32 * h + 32, 0:N],
                            vsb[r][:, kt * 128 + 32 * h:kt * 128 + 32 * h + 32],
                            pes[r][:, (kt * H + h) * N:(kt * H + h + 1) * N],
                            start=(kt == 0), stop=(kt == 1),
                            tile_position=(0, 32 * h), skip_group_check=True)
                for kt in range(2):
                    for h in range(H):
                        nc.tensor.matmul(
                            psOD[32 * h:32 * h + 32, N:2 * N], sel_s[:],
                            pes[r][:, (kt * H + h) * N:(kt * H + h + 1) * N],
                            start=(kt == 0), stop=(kt == 1),
                            tile_position=(0, 32 * h), skip_group_check=True)
                return psOD

            def og_a(r, psOD):
                # og = (tanh + 1) * psO  (gating, 2*sigmoid fold)
                ogs[r] = sb.tile([128, N], fp16, tag="og", name="og")
                nc.vector.scalar_tensor_tensor(
                    ogs[r][:], tts[r][:], 1.0, psOD[:, 0:N],
                    op0=ALU.add, op1=ALU.mult)

            def og_b(r, psOD):
                rbs[r] = sb.tile([128, N], f32, tag="rb", name="rb")
                nc.vector.reciprocal_approx_fast(rbs[r][:], psOD[:, N:2 * N])

            def og_c(r):
                og2s[r] = sb.tile([128, N], fp16, tag="og2", name="og2")
                nc.vector.scalar_tensor_tensor(
                    og2s[r][:], ogs[r][:], 1.0, rbs[r][:],
                    op0=ALU.mult, op1=ALU.mult)

            def fin_stage(r, psF):
                # bo is added by the ocopy STT; no bias matmuls
                reg = (r % 2) * N
                for qt in range(2):
                    nc.tensor.matmul(psF[:, reg + qt * 128:reg + (qt + 1) * 128],
                                     og2s[r][:, qt * 128:(qt + 1) * 128], wo_s[:],
                                     start=True, stop=True)

            # ---- software pipeline ----
            loads(0)
            loads(1)
            loads(2)
            u_stage(0, 0)
            u_stage(0, 1)
            psF = None
            for i in range(rows + 2):
                r_load, r_next, r_cur, r_av, r_fin = i + 3, i + 1, i, i - 1, i - 2
                if r_load < rows:
                    loads(r_load)
                if r_next < rows:
                    u_stage(r_next, 0)  # PE u_kt0 + DVE copy half
                if r_cur < rows:
                    vg_stage(r_cur)     # PE vg + ACT tanh + DVE vcopy
                    scores_stage(r_cur, 0)
                if 0 <= r_av < rows:
                    psOD = av_stage(r_av)   # PE AV + den (pE ready last iter)
                    og_a(r_av, psOD)        # DVE og
                if r_cur < rows:
                    mult_stage(r_cur, 0)    # DVE pE kt0 (after exp0 only)
                if r_next < rows:
                    u_stage(r_next, 1)  # PE u_kt1 + ACT copy half
                if 0 <= r_av < rows:
                    og_b(r_av, psOD)        # DVE rb
                if r_cur < rows:
                    scores_stage(r_cur, 1)
                if 0 <= r_av < rows:
                    og_c(r_av)              # DVE og2
                if r_cur < rows:
                    mult_stage(r_cur, 1)    # DVE pE kt1
                if 0 <= r_fin < rows:
                    if r_fin % 2 == 0:
                        psF = ps.tile([128, 512], f32, tag="psF")
                    fin_stage(r_fin, psF)
                    if r_fin % 2 == 1:
                        o_sb = sb.tile([128, 512], fp16, tag="osb")
                        nc.vector.scalar_tensor_tensor(
                            o_sb[:], psF[:], 1.0, bo_bc[:],
                            op0=ALU.mult, op1=ALU.add)
                        for rr in range(2):
                            n = r_fin - 1 + rr
                            nc.gpsimd.dma_start(
                                out[n].rearrange("(t p) c -> p t c", p=128),
                                o_sb[:, rr * N:(rr + 1) * N]
                                .rearrange("p (t c) -> p t c", c=128))
    nc.compile()
    return nc


_PROG_CACHE = {}


def host_prep(q_x, kv_x, mask_bias, triangle_bias, wq, wk, wv, wg, bg, wo, bo):
    """Returns (qkx fp16 [N,2N,C], maskc [N,128,2], shared dict)."""
    scale = np.float64(1.0 / np.float64(np.sqrt(np.float32(CH), dtype=np.float32)))
    qkx = np.concatenate(
        [np.asarray(q_x, np.float32).reshape(N, N, CQ).astype(np.float16),
         np.asarray(kv_x, np.float32).reshape(N, N, CQ).astype(np.float16)],
        axis=1)  # [N, 2N, CQ]

    wqf = np.asarray(wq, np.float64).reshape(H, CH, CQ)
    wkf = np.asarray(wk, np.float64).reshape(H, CH, CQ)
    # M_h = wk_h.T @ wq_h * scale * 256 (x256 dodges fp16 subnormals;
    # exp's scale=1/256 compensates), mcat [c, h*CQ + c']
    mcat = np.concatenate(
        [(wkf[h].T @ wqf[h] * (scale * 256.0)) for h in range(H)],
        axis=1).astype(np.float16)
    mcat = np.ascontiguousarray(mcat)
    wvT = np.ascontiguousarray(np.asarray(wv).reshape(HD, CQ).T.astype(np.float16))
    wgT = np.ascontiguousarray(np.asarray(wg).reshape(HD, CQ).T.astype(np.float16))
    woT = np.ascontiguousarray(np.asarray(wo).T.astype(np.float16))  # [e, c]
    bgc = np.ascontiguousarray(np.asarray(bg, np.float32).reshape(HD, 1) * 0.5)
    # bo broadcast to the 2-row psF layout [128, (r, qt, c)]
    bor = np.ascontiguousarray(
        np.tile(np.asarray(bo).astype(np.float16).reshape(1, CQ), (128, 4)))
    sele = np.full((128, 32), 2.0, np.float16)
    # mask: [n, k] -> [n, k_in_tile, kt]
    m = np.asarray(mask_bias, np.float32).reshape(N, N)
    maskc = np.ascontiguousarray(m.reshape(N, 2, 128).transpose(0, 2, 1))
    # E = exp(tri): [h, q, k] -> [(kt, h), k_in_tile, q], fp16
    t = np.asarray(triangle_bias, np.float64).reshape(H, N, N)
    ecat = np.ascontiguousarray(
        np.exp(t).transpose(0, 2, 1).reshape(H, 2, 128, N).transpose(1, 0, 2, 3)
        .reshape(2 * H, 128, N).astype(np.float16))
    shared = dict(mcat=mcat, wvT=wvT, wgT=wgT, woT=woT, bgc=bgc,
                  bor=bor, sele=sele, ecat=ecat)
    return qkx, maskc, shared


def make_in_maps(q_x, kv_x, mask_bias, triangle_bias, wq, wk, wv, wg, bg, wo, bo):
    qkx, maskc, shared = host_prep(q_x, kv_x, mask_bias, triangle_bias,
                                   wq, wk, wv, wg, bg, wo, bo)
    in_maps = []
    for i in range(NCORES):
        sl = slice(i * ROWS, (i + 1) * ROWS)
        in_maps.append(dict(qkx=np.ascontiguousarray(qkx[sl]),
                            maskc=np.ascontiguousarray(maskc[sl]), **shared))
    return in_maps


def get_program():
    if ROWS not in _PROG_CACHE:
        _PROG_CACHE[ROWS] = build_program(ROWS)
    return _PROG_CACHE[ROWS]


def kernel(q_x, kv_x, mask_bias, triangle_bias, wq, wk, wv, wg, bg, wo, bo):
    from concourse.bass_utils import run_bass_kernel_spmd

    in_maps = make_in_maps(q_x, kv_x, mask_bias, triangle_bias,
                           wq, wk, wv, wg, bg, wo, bo)
    nc = get_program()
    res = run_bass_kernel_spmd(nc, in_maps, list(range(NCORES)))
    outs = [np.asarray(res.results[i]["out"]) for i in range(NCORES)]
    return np.concatenate(outs, axis=0)[None].astype(np.float32)


# revision 15
# speedup vs baseline: 1.4242x; 1.1388x over previous
"""Trainium2 Bass kernel for triangle (AlphaFold-style) gated attention over pair rows.

Problem: B=1, N=256 rows; per row n: attention over 256 positions,
H=4 heads x CH=32, C=128 channels, additive mask bias (per row, per key),
triangle bias (per head, q, k; shared across rows), sigmoid gating,
output projection. Rows sharded across 8 NeuronCores (32 rows/core), SPMD.

Design (vs v1 baseline at 195us):
  - triangle bias applied MULTIPLICATIVELY: exp(s+tri+mask) =
    exp(s+mask) * exp(tri); E = exp(tri) is row-constant, precomputed on
    host in fp16, applied by per-kt DVE tensor_mul (2x fp16 mode).
    Kills the per-row identity-matmul PSUM preload (2048 PE cyc/row).
  - inputs pre-cast to fp16 on host, qx|kvx concatenated; ONE row load
    via DMA xbar transpose (dma_start_transpose DRAM->SBUF) into
    [c, tok] layout. Kills gpsimd cast loads, PE transposes, DVE copies.
  - scores via host-precomputed M_h = wk_h.T wq_h * scale * 256 (fp16):
    u = M.T @ xkT (k-tile chunks), sT_h = u_slice.T @ xqT (all K=128).
  - p = exp(psS/256 + mask) per kt chunk (ACT, mask = per-partition
    bias); pE = p * E per-kt DVE tensor_mul so AV kt0 never waits exp1.
  - AV + denominators via tile_position column packing (4-way
    concurrent on HW), sele=2.0 broadcast; gating fused:
    og = (tanh+1)*psO in one STT op; bo folded into the output-copy STT.
  - output stores on gpsimd SWDGE queue (keeps the sync queue, which
    serially executes the 1.2us DMA transposes, from blocking loads).
  - software pipeline: loads(i+3) / u(i+1) / main(i) / AV(i-1) /
    fin(i-2, 2-row batched); PSUM = 8 banks exactly:
    psU 1, psB 1, psS0 2, psS1 2, psOD 1, psF 1.
"""
import numpy as np

B, N, CQ, H, CH = 1, 256, 128, 4, 32
NCORES = 8
ROWS = N // NCORES  # 32
HD = H * CH  # 128


def build_program(rows):
    import concourse.bass as bass
    import concourse.bacc as bacc
    import concourse.mybir as mybir
    from concourse import tile

    f32 = mybir.dt.float32
    fp16 = mybir.dt.float16
    AF = mybir.ActivationFunctionType
    ALU = mybir.AluOpType
    nc = bacc.Bacc("TRN2", target_bir_lowering=False, debug=False)

    qkx = nc.declare_dram_parameter("qkx", [rows, 2 * N, CQ], fp16,
                                    isOutput=False)
    maskc = nc.declare_dram_parameter("maskc", [rows, 128, 2], f32, isOutput=False)
    ecat = nc.declare_dram_parameter("ecat", [2 * H, 128, N], fp16, isOutput=False)
    mcat = nc.declare_dram_parameter("mcat", [CQ, H * CQ], fp16, isOutput=False)
    wvT = nc.declare_dram_parameter("wvT", [CQ, HD], fp16, isOutput=False)
    wgT = nc.declare_dram_parameter("wgT", [CQ, HD], fp16, isOutput=False)
    woT = nc.declare_dram_parameter("woT", [HD, CQ], fp16, isOutput=False)
    bgc = nc.declare_dram_parameter("bgc", [HD, 1], f32, isOutput=False)
    bor = nc.declare_dram_parameter("bor", [128, 4 * CQ], fp16, isOutput=False)
    sele = nc.declare_dram_parameter("sele", [128, 32], fp16, isOutput=False)
    out = nc.declare_dram_parameter("out", [rows, N, CQ], fp16, isOutput=True)

    with tile.TileContext(nc) as tc:
        with (
            nc.allow_low_precision(reason="fp16 matmul operands and "
                                   "reciprocal_approx_fast by design"),
            tc.tile_pool(name="const", bufs=1) as cp,
            tc.tile_pool(name="sb", bufs=3) as sb,
            tc.tile_pool(name="sbp", bufs=6) as sbp,
            tc.tile_pool(name="ps", bufs=1, space=bass.MemorySpace.PSUM) as ps,
        ):
            # ---- constants ----
            m_s = cp.tile([CQ, H * CQ], fp16, tag="mcat")
            wv_s = cp.tile([CQ, HD], fp16, tag="wv")
            wg_s = cp.tile([CQ, HD], fp16, tag="wg")
            wo_s = cp.tile([HD, CQ], fp16, tag="wo")
            bg_s = cp.tile([HD, 1], f32, tag="bg")
            bo_bc = cp.tile([128, 4 * CQ], fp16, tag="bo")
            sel_s = cp.tile([128, 32], fp16, tag="sele")
            e_s = cp.tile([128, 2 * H * N], fp16, tag="ecat")
            mk_all = cp.tile([128, rows, 2], f32, tag="mkall")
            for t, d in ((m_s, mcat), (wv_s, wvT), (wg_s, wgT), (wo_s, woT),
                         (bg_s, bgc), (bo_bc, bor), (sel_s, sele)):
                nc.sync.dma_start(t[:], d[:])
            for i in range(2 * H):
                nc.sync.dma_start(e_s[:, i * N:(i + 1) * N], ecat[i])
            nc.sync.dma_start(mk_all[:], maskc.rearrange("r p t -> p r t"))

            # per-row tiles live in dicts so the software pipeline below can
            # reference them by row index
            xq = {}
            xk = {}
            usb = {}
            vsb = {}
            tts = {}
            pss = {}
            pes = {}
            rbs = {}
            ogs = {}
            og2s = {}

            def loads(r):
                xqk = sbp.tile([CQ, 2 * N], fp16, tag="xqk", name="xqk")
                nc.sync.dma_start_transpose(xqk[:], qkx[r])
                xq[r] = xqk[:, 0:N]
                xk[r] = xqk[:, N:2 * N]

            def u_stage(r, kt):
                # u_h[c', k-tile] for 4 heads of this kt -> psU [128, 512]
                psU = ps.tile([128, 512], f32, tag="psU")
                for h in range(H):
                    nc.tensor.matmul(psU[:, h * 128:(h + 1) * 128],
                                     m_s[:, h * CQ:(h + 1) * CQ],
                                     xk[r][:, kt * 128:(kt + 1) * 128],
                                     start=True, stop=True)
                if kt == 0:
                    usb[r] = sb.tile([CQ, 1024], fp16, tag="usb", name="usb")
                    nc.vector.tensor_copy(usb[r][:, 0:512], psU[:])
                else:
                    nc.scalar.activation(usb[r][:, 512:1024], psU[:], AF.Copy)

            def vg_stage(r):
                psB = ps.tile([128, 512], f32, tag="psB")  # g [0:256] | v [256:512]
                nc.tensor.matmul(psB[:, 0:N], wg_s[:], xq[r],
                                 start=True, stop=True)
                for kt in range(2):
                    nc.tensor.matmul(psB[:, N + kt * 128:N + (kt + 1) * 128],
                                     xk[r][:, kt * 128:(kt + 1) * 128], wv_s[:],
                                     start=True, stop=True)
                tts[r] = sb.tile([128, N], fp16, tag="tT", name="tT")
                nc.scalar.activation(tts[r][:], psB[:, 0:N], AF.Tanh,
                                     scale=0.5, bias=bg_s[:, 0:1])
                vsb[r] = sb.tile([128, N], fp16, tag="v", name="vsb")
                nc.vector.tensor_copy(vsb[r][:], psB[:, N:2 * N])

            def scores_stage(r, kt):
                psS = ps.tile([128, H * N], f32, tag=f"psS{kt}")
                for h in range(H):
                    nc.tensor.matmul(
                        psS[:, h * N:(h + 1) * N],
                        usb[r][:, kt * 512 + h * 128:kt * 512 + (h + 1) * 128],
                        xq[r], start=True, stop=True)
                if kt == 0:
                    pss[r] = sb.tile([128, 2 * H * N], fp16, tag="ps", name="ps")
                nc.scalar.activation(pss[r][:, kt * H * N:(kt + 1) * H * N],
                                     psS[:], AF.Exp, scale=float(1.0 / 256.0),
                                     bias=mk_all[:, r, kt:kt + 1])

            def mult_stage(r, kt):
                if kt == 0:
                    pes[r] = sb.tile([128, 2 * H * N], fp16, tag="pe", name="pe")
                sl = slice(kt * H * N, (kt + 1) * H * N)
                nc.vector.tensor_mul(pes[r][:, sl], pss[r][:, sl], e_s[:, sl])

            def av_stage(r):
                psOD = ps.tile([128, 512], f32, tag="psOD")  # o [0:256]|den[256:512]
                for kt in range(2):
                    for h in range(H):
                        nc.tensor.matmul(
                            psOD[32 * h:32 * h + 32, 0:N],
                            vsb[r][:, kt * 128 + 32 * h:kt * 128 + 32 * h + 32],
                            pes[r][:, (kt * H + h) * N:(kt * H + h + 1) * N],
                            start=(kt == 0), stop=(kt == 1),
                            tile_position=(0, 32 * h), skip_group_check=True)
                for kt in range(2):
                    for h in range(H):
                        nc.tensor.matmul(
                            psOD[32 * h:32 * h + 32, N:2 * N], sel_s[:],
                            pes[r][:, (kt * H + h) * N:(kt * H + h + 1) * N],
                            start=(kt == 0), stop=(kt == 1),
                            tile_position=(0, 32 * h), skip_group_check=True)
                return psOD

            def og_a(r, psOD):
                # og = (tanh + 1) * psO  (gating, 2*sigmoid fold)
                ogs[r] = sb.tile([128, N], fp16, tag="og", name="og")
                nc.vector.scalar_tensor_tensor(
                    ogs[r][:], tts[r][:], 1.0, psOD[:, 0:N],
                    op0=ALU.add, op1=ALU.mult)

            def og_b(r, psOD):
                rbs[r] = sb.tile([128, N], f32, tag="rb", name="rb")
                nc.vector.reciprocal_approx_fast(rbs[r][:], psOD[:, N:2 * N])

            def og_c(r):
                og2s[r] = sb.tile([128, N], fp16, tag="og2", name="og2")
                nc.gpsimd.tensor_mul(og2s[r][:], ogs[r][:], rbs[r][:])

            def fin_stage(r, psF):
                # bo is added by the ocopy STT; no bias matmuls
                reg = (r % 2) * N
                for qt in range(2):
                    nc.tensor.matmul(psF[:, reg + qt * 128:reg + (qt + 1) * 128],
                                     og2s[r][:, qt * 128:(qt + 1) * 128], wo_s[:],
                                     start=True, stop=True)

            # ---- software pipeline ----
            loads(0)
            loads(1)
            loads(2)
            loads(3)
            u_stage(0, 0)
            u_stage(0, 1)
            u_stage(1, 0)
            u_stage(1, 1)
            vg_stage(0)
            psF = None
            for i in range(rows + 2):
                r_load, r_u, r_vg, r_cur, r_av, r_fin = (
                    i + 4, i + 2, i + 1, i, i - 1, i - 2)
                if r_load < rows:
                    loads(r_load)
                if r_cur < rows:
                    scores_stage(r_cur, 0)
                if r_u < rows:
                    u_stage(r_u, 0)     # PE u_kt0 + DVE copy half
                if r_vg < rows:
                    vg_stage(r_vg)      # PE vg + ACT tanh + DVE vcopy
                if 0 <= r_av < rows:
                    psOD = av_stage(r_av)   # PE AV + den (pE ready last iter)
                    og_a(r_av, psOD)        # DVE og
                if r_cur < rows:
                    mult_stage(r_cur, 0)    # DVE pE kt0 (after exp0 only)
                if r_u < rows:
                    u_stage(r_u, 1)     # PE u_kt1 + ACT copy half
                if 0 <= r_av < rows:
                    og_b(r_av, psOD)        # DVE rb
                if r_cur < rows:
                    scores_stage(r_cur, 1)
                if 0 <= r_av < rows:
                    og_c(r_av)              # gpsimd og2
                if r_cur < rows:
                    mult_stage(r_cur, 1)    # DVE pE kt1
                if 0 <= r_fin < rows:
                    if r_fin % 2 == 0:
                        psF = ps.tile([128, 512], f32, tag="psF")
                    fin_stage(r_fin, psF)
                    if r_fin % 2 == 1:
                        o_sb = sb.tile([128, 512], fp16, tag="osb")
                        nc.vector.scalar_tensor_tensor(
                            o_sb[:], psF[:], 1.0, bo_bc[:],
                            op0=ALU.mult, op1=ALU.add)
                        for rr in range(2):
                            n = r_fin - 1 + rr
                            nc.sync.dma_start(
                                out[n].rearrange("(t p) c -> p t c", p=128),
                                o_sb[:, rr * N:(rr + 1) * N]
                                .rearrange("p (t c) -> p t c", c=128))
    nc.compile()
    return nc


_PROG_CACHE = {}


def host_prep(q_x, kv_x, mask_bias, triangle_bias, wq, wk, wv, wg, bg, wo, bo):
    """Returns (qkx fp16 [N,2N,C], maskc [N,128,2], shared dict)."""
    scale = np.float64(1.0 / np.float64(np.sqrt(np.float32(CH), dtype=np.float32)))
    qkx = np.concatenate(
        [np.asarray(q_x, np.float32).reshape(N, N, CQ).astype(np.float16),
         np.asarray(kv_x, np.float32).reshape(N, N, CQ).astype(np.float16)],
        axis=1)  # [N, 2N, CQ]

    wqf = np.asarray(wq, np.float64).reshape(H, CH, CQ)
    wkf = np.asarray(wk, np.float64).reshape(H, CH, CQ)
    # M_h = wk_h.T @ wq_h * scale * 256 (x256 dodges fp16 subnormals;
    # exp's scale=1/256 compensates), mcat [c, h*CQ + c']
    mcat = np.concatenate(
        [(wkf[h].T @ wqf[h] * (scale * 256.0)) for h in range(H)],
        axis=1).astype(np.float16)
    mcat = np.ascontiguousarray(mcat)
    wvT = np.ascontiguousarray(np.asarray(wv).reshape(HD, CQ).T.astype(np.float16))
    wgT = np.ascontiguousarray(np.asarray(wg).reshape(HD, CQ).T.astype(np.float16))
    woT = np.ascontiguousarray(np.asarray(wo).T.astype(np.float16))  # [e, c]
    bgc = np.ascontiguousarray(np.asarray(bg, np.float32).reshape(HD, 1) * 0.5)
    # bo broadcast to the 2-row psF layout [128, (r, qt, c)]
    bor = np.ascontiguousarray(
        np.tile(np.asarray(bo).astype(np.float16).reshape(1, CQ), (128, 4)))
    sele = np.full((128, 32), 2.0, np.float16)
    # mask: [n, k] -> [n, k_in_tile, kt]
    m = np.asarray(mask_bias, np.float32).reshape(N, N)
    maskc = np.ascontiguousarray(m.reshape(N, 2, 128).transpose(0, 2, 1))
    # E = exp(tri): [h, q, k] -> [(kt, h), k_in_tile, q], fp16
    t = np.asarray(triangle_bias, np.float64).reshape(H, N, N)
    ecat = np.ascontiguousarray(
        np.exp(t).transpose(0, 2, 1).reshape(H, 2, 128, N).transpose(1, 0, 2, 3)
        .reshape(2 * H, 128, N).astype(np.float16))
    shared = dict(mcat=mcat, wvT=wvT, wgT=wgT, woT=woT, bgc=bgc,
                  bor=bor, sele=sele, ecat=ecat)
    return qkx, maskc, shared


def make_in_maps(q_x, kv_x, mask_bias, triangle_bias, wq, wk, wv, wg, bg, wo, bo):
    qkx, maskc, shared = host_prep(q_x, kv_x, mask_bias, triangle_bias,
                                   wq, wk, wv, wg, bg, wo, bo)
    in_maps = []
    for i in range(NCORES):
        sl = slice(i * ROWS, (i + 1) * ROWS)
        in_maps.append(dict(qkx=np.ascontiguousarray(qkx[sl]),
                            maskc=np.ascontiguousarray(maskc[sl]), **shared))
    return in_maps


def get_program():
    if ROWS not in _PROG_CACHE:
        _PROG_CACHE[ROWS] = build_program(ROWS)
    return _PROG_CACHE[ROWS]


def kernel(q_x, kv_x, mask_bias, triangle_bias, wq, wk, wv, wg, bg, wo, bo):
    from concourse.bass_utils import run_bass_kernel_spmd

    in_maps = make_in_maps(q_x, kv_x, mask_bias, triangle_bias,
                           wq, wk, wv, wg, bg, wo, bo)
    nc = get_program()
    res = run_bass_kernel_spmd(nc, in_maps, list(range(NCORES)))
    outs = [np.asarray(res.results[i]["out"]) for i in range(NCORES)]
    return np.concatenate(outs, axis=0)[None].astype(np.float32)
